# revision 10
# baseline (speedup 1.0000x reference)
"""Single-head encoder attention block on 8 Trainium2 NeuronCores.

Math (per batch element b):
    q = x @ wq.T ; k = x @ wk.T ; v = x @ wv.T
    scores = (q @ k.T) / sqrt(1024) ; attn = softmax(scores, -1)
    out = (attn @ v) @ wo.T

Sharding: data-parallel over batch — batch 8 maps 1:1 onto the 8 cores;
weights replicated. No collectives.

Per-core algorithm (storage fp32 bits; matmul operands typed MM_DT):
  Two weight-product folds remove almost all operand transposes:
      scores = x (wq.T wk) x.T / 32            M  := wq.T @ wk
      attn @ v @ wo.T = attn @ x @ (wo wv).T   via UT[d,do] = sum_vc wv[vc,d] woT[vc,do]
  Phase A (ordered so DMA/PE never idle at phase seams):
    A0: xT via identity-matmul transposes (resident)  [1024d, 2048s]
    A1: woT (identity-matmul transposes, do-halves), UT = wv-nat x woT,
        Z = xT-stationary x UT (resident)             [2048j, 1024do]
    A2: M = wq.T @ wk (wk resident, wq column-sliced) [1024d1, 1024d2]
    A3: F = (M @ xT)/32 spilled to DRAM               [1024d2, 2048i]
  Phase B (per i-superblock of SB=512):
    scoresT[j,i] = sum_d2 xT[d2,j]*F[d2,i]; expT = exp(scoresT)
    rowsum over j via ones-matmul; reciprocal; transposed to per-partition
    out[i,do] = (sum_j expT[j,i-tile] * Z[j,do]) * recip[i]   (expT stationary)
"""

import os
import sys

for _p in ("/opt/trn_rl_repo", "/root/.axon_site/_ro/trn_rl_repo"):
    if os.path.isdir(_p) and _p not in sys.path:
        sys.path.insert(0, _p)

import numpy as np
from contextlib import ExitStack

import concourse.bacc as bacc
import concourse.tile as tile
from concourse import mybir, masks
from concourse.bass_utils import run_bass_kernel_spmd

P = 128
S = 2048          # sequence length (per core)
D = 1024          # model dim = dk = dv
NS = S // P       # 16 seq tiles
ND = D // P       # 8 dim tiles
SB = 512          # i-superblock width (query columns per block)
NSB = S // SB     # 4 superblocks
NIT = SB // P     # 4 i-tiles per superblock
SCALE = 1.0 / 32.0  # 1/sqrt(1024)
N_CORES = 8

DT = mybir.dt.float32
MM_DT = mybir.dt.float32r if os.environ.get("ENC_MM_DT", "f32r") == "f32r" else mybir.dt.float32
F32 = mybir.dt.float32
EXP = mybir.ActivationFunctionType.Exp
COPY = mybir.ActivationFunctionType.Copy


def _build():
    nc = bacc.Bacc("TRN2", target_bir_lowering=False, debug=False, num_devices=N_CORES)

    x_in = nc.dram_tensor("x", [S, D], DT, kind="ExternalInput").ap()
    wq_in = nc.dram_tensor("wq", [D, D], DT, kind="ExternalInput").ap()
    wk_in = nc.dram_tensor("wk", [D, D], DT, kind="ExternalInput").ap()
    wv_in = nc.dram_tensor("wv", [D, D], DT, kind="ExternalInput").ap()
    wo_in = nc.dram_tensor("wo", [D, D], DT, kind="ExternalInput").ap()
    out_d = nc.dram_tensor("out", [S, D], DT, kind="ExternalOutput").ap()
    f_dram = nc.dram_tensor("f_scratch", [D, S], MM_DT).ap()

    mm = nc.tensor.matmul

    with tile.TileContext(nc) as tc, ExitStack() as top:
        cst = top.enter_context(tc.tile_pool(name="cst", bufs=1))
        ident_f32 = cst.tile([P, P], DT)
        masks.make_identity(nc, ident_f32[:])
        ident = cst.tile([P, P], MM_DT)
        nc.vector.tensor_copy(ident[:], ident_f32[:])
        ones_f32 = cst.tile([P, 1], DT)
        nc.gpsimd.memset(ones_f32[:], 1.0)
        ones = cst.tile([P, 1], MM_DT)
        nc.vector.tensor_copy(ones[:], ones_f32[:])

        def tr(out_ap, in_ap):
            """out_ap[PSUM 128x128] = in_ap.T via normal matmul against identity."""
            mm(out_ap, in_ap, ident[:], start=True, stop=True)

        res1 = top.enter_context(tc.tile_pool(name="res1", bufs=1))
        xt = res1.tile([P, ND * S], MM_DT)    # xT: tile d -> [:, d*S:(d+1)*S] = [d-part, s]
        res2 = top.enter_context(tc.tile_pool(name="res2", bufs=1))
        zres = res2.tile([P, NS * D], MM_DT)  # Z: tile j -> [:, j*D:(j+1)*D] = [j-part, do]

        # ---------------- Phase A0/A1: xT, woT, UT, Z ----------------
        with ExitStack() as pw:
            tpps = pw.enter_context(tc.tile_pool(name="tpps", bufs=3, space="PSUM"))
            mmps = pw.enter_context(tc.tile_pool(name="mmps", bufs=5, space="PSUM"))
            ldp = pw.enter_context(tc.tile_pool(name="ldp", bufs=3))
            wvp = pw.enter_context(tc.tile_pool(name="wvp", bufs=1))
            hwork = pw.enter_context(tc.tile_pool(name="hwork", bufs=1))

            # wv natural loads kick off immediately (overlap x transposes)
            wvn = wvp.tile([P, ND * D], MM_DT)   # wv natural: vc-tile t -> [:, t*D:(t+1)*D]
            for t in range(ND):
                nc.sync.dma_start(out=wvn[:, t * D:(t + 1) * D], in_=wv_in[t * P:(t + 1) * P, :].bitcast(MM_DT))

            # A0: load x row-tiles, transpose into xT
            for s in range(NS):
                xs = ldp.tile([P, D], MM_DT, tag="ld")
                nc.sync.dma_start(out=xs[:], in_=x_in[s * P:(s + 1) * P, :].bitcast(MM_DT))
                for d in range(ND):
                    tp = tpps.tile([P, P], F32, tag="tp")
                    tr(tp[:], xs[:, d * P:(d + 1) * P])
                    nc.vector.tensor_copy(xt[:, d * S + s * P: d * S + (s + 1) * P], tp[:])

            # A1: per do-half: woT half, UT half, Z half
            for h in range(2):
                wot_h = hwork.tile([P, ND * 512], MM_DT, name=f"woth{h}", tag="wot")
                for dot in range(4):   # do-tiles within the half
                    wn = ldp.tile([P, D], MM_DT, tag="ld")
                    do_row = h * 4 + dot
                    nc.sync.dma_start(out=wn[:], in_=wo_in[do_row * P:(do_row + 1) * P, :].bitcast(MM_DT))
                    for vc in range(ND):
                        tp = tpps.tile([P, P], F32, tag="tp")
                        tr(tp[:], wn[:, vc * P:(vc + 1) * P])
                        nc.vector.tensor_copy(
                            wot_h[:, vc * 512 + dot * P: vc * 512 + (dot + 1) * P], tp[:])
                ut_h = hwork.tile([P, ND * 512], MM_DT, name=f"uth{h}", tag="ut")
                for d in range(ND):
                    ps = mmps.tile([P, 512], F32, tag="mm")
                    for vc in range(ND):
                        mm(ps[:],
                           wvn[:, vc * D + d * P: vc * D + (d + 1) * P],
                           wot_h[:, vc * 512:(vc + 1) * 512],
                           start=(vc == 0), stop=(vc == ND - 1))
                    nc.scalar.copy(ut_h[:, d * 512:(d + 1) * 512], ps[:])
                for j in range(NS):
                    ps = mmps.tile([P, 512], F32, tag="mm")
                    for d in range(ND):
                        mm(ps[:],
                           xt[:, d * S + j * P: d * S + (j + 1) * P],
                           ut_h[:, d * 512:(d + 1) * 512],
                           start=(d == 0), stop=(d == ND - 1))
                    nc.scalar.copy(zres[:, j * D + h * 512: j * D + (h + 1) * 512], ps[:])

        # ---------------- Phase A2/A3: M then F ----------------
        with ExitStack() as pa:
            mmps2 = pa.enter_context(tc.tile_pool(name="mmps2", bufs=6, space="PSUM"))
            ldq = pa.enter_context(tc.tile_pool(name="ldq", bufs=4))
            evp = pa.enter_context(tc.tile_pool(name="evp", bufs=4))
            wkp = pa.enter_context(tc.tile_pool(name="wkp", bufs=1))
            mwork = pa.enter_context(tc.tile_pool(name="mwork", bufs=1))

            wkn = wkp.tile([P, ND * D], MM_DT)
            mres = mwork.tile([P, ND * D], MM_DT)  # M d1-tile -> [:, d1*D:(d1+1)*D] = [d1-part, d2]
            for t in range(ND):
                nc.sync.dma_start(out=wkn[:, t * D:(t + 1) * D], in_=wk_in[t * P:(t + 1) * P, :].bitcast(MM_DT))

            # A2: M = wq.T @ wk; wq streamed as [128, 256] column slices
            for q in range(4):           # d1-pairs
                pq = [mmps2.tile([P, 512], F32, name=f"mq{i}", tag="mm") for i in range(4)]
                for ct in range(ND):
                    wqs = ldq.tile([P, 256], MM_DT, tag="wq")
                    nc.sync.dma_start(
                        out=wqs[:],
                        in_=wq_in[ct * P:(ct + 1) * P, q * 256:(q + 1) * 256].bitcast(MM_DT))
                    for dl in range(2):
                        for ch in range(2):
                            mm(pq[dl * 2 + ch][:],
                               wqs[:, dl * P:(dl + 1) * P],
                               wkn[:, ct * D + ch * 512: ct * D + (ch + 1) * 512],
                               start=(ct == 0), stop=(ct == ND - 1))
                for dl in range(2):
                    for ch in range(2):
                        d1 = q * 2 + dl
                        nc.scalar.copy(mres[:, d1 * D + ch * 512: d1 * D + (ch + 1) * 512],
                                       pq[dl * 2 + ch][:])

            # A3: F[d2,i] = sum_d1 M[d1,d2] xT[d1,i], scaled by 1/32, spilled
            for d2 in range(ND):
                pss = [mmps2.tile([P, 512], F32, name=f"fps{ic}", tag="mm") for ic in range(4)]
                for d1 in range(ND):
                    for ic in range(4):
                        mm(pss[ic][:],
                           mres[:, d1 * D + d2 * P: d1 * D + (d2 + 1) * P],
                           xt[:, d1 * S + ic * 512: d1 * S + (ic + 1) * 512],
                           start=(d1 == 0), stop=(d1 == ND - 1))
                for ic in range(4):
                    ev = evp.tile([P, 512], MM_DT, tag="ev")
                    nc.scalar.mul(ev[:], pss[ic][:], SCALE)
                    nc.sync.dma_start(out=f_dram[d2 * P:(d2 + 1) * P, ic * 512:(ic + 1) * 512], in_=ev[:])

        # ---------------- Phase B ----------------
        with ExitStack() as pb:
            scps = pb.enter_context(tc.tile_pool(name="scps", bufs=3, space="PSUM"))
            outps = pb.enter_context(tc.tile_pool(name="outps", bufs=3, space="PSUM"))
            miscps = pb.enter_context(tc.tile_pool(name="miscps", bufs=2, space="PSUM"))
            fbp = pb.enter_context(tc.tile_pool(name="fbp", bufs=10))
            expp = pb.enter_context(tc.tile_pool(name="expp", bufs=16))
            outsb = pb.enter_context(tc.tile_pool(name="outsb", bufs=3))
            rsp = pb.enter_context(tc.tile_pool(name="rsp", bufs=2))
            rtp_pool = pb.enter_context(tc.tile_pool(name="rtp_pool", bufs=6))

            for sbi in range(NSB):
                fb = []
                for d2 in range(ND):
                    f = fbp.tile([P, SB], MM_DT, name=f"fb{d2}", tag="fb")
                    nc.sync.dma_start(out=f[:], in_=f_dram[d2 * P:(d2 + 1) * P, sbi * SB:(sbi + 1) * SB])
                    fb.append(f)

                # scoresT + exp per j-tile
                ets = []
                for j in range(NS):
                    sc = scps.tile([P, SB], F32, tag="sc")
                    for d2 in range(ND):
                        mm(sc[:],
                           xt[:, d2 * S + j * P: d2 * S + (j + 1) * P],
                           fb[d2][:],
                           start=(d2 == 0), stop=(d2 == ND - 1))
                    et = expp.tile([P, SB], MM_DT, name=f"et{j}", tag="et")
                    nc.scalar.activation(et[:], sc[:], EXP)
                    ets.append(et)

                # rowsums over j (partition dim) via ones-matmul
                rs = miscps.tile([1, SB], F32, tag="m")
                for j in range(NS):
                    mm(rs[:], ones[:, 0:1], ets[j][:], start=(j == 0), stop=(j == NS - 1))

                # reciprocal chain (DVE) — emitted early so it overlaps out-MMs
                rs_sb = rsp.tile([1, SB], DT, tag="rs")
                nc.vector.tensor_copy(rs_sb[:], rs[:])
                rc_sb = rsp.tile([1, SB], DT, tag="rc")
                nc.vector.reciprocal(rc_sb[:], rs_sb[:])

                # out[i,do] = sum_j expT[j, i-tile].T @ Z[j, do-chunk]; evict fused
                recips = [None] * NIT
                for gi in range(NIT * 2):
                    it, ch = gi // 2, gi % 2
                    op = outps.tile([P, 512], F32, name=f"op{ch}", tag="op")
                    for j in range(NS):
                        mm(op[:],
                           ets[j][:, it * P:(it + 1) * P],
                           zres[:, j * D + ch * 512: j * D + (ch + 1) * 512],
                           start=(j == 0), stop=(j == NS - 1))
                    if gi == 0:
                        # per-partition recip tiles via tiny PE transposes; PE
                        # reaches these after group 0 while DVE chain is done
                        for it2 in range(NIT):
                            tp = miscps.tile([P, 1], F32, name=f"rtp{it2}", tag="m")
                            nc.tensor.transpose(tp[:], rc_sb[:1, it2 * P:(it2 + 1) * P], ident_f32[:1, :1])
                            rt = rtp_pool.tile([P, 1], DT, name=f"rt{it2}", tag="rt")
                            nc.vector.tensor_copy(rt[:], tp[:])
                            recips[it2] = rt
                    ob = outsb.tile([P, 512], DT, tag="ob")
                    nc.scalar.activation(ob[:], op[:], COPY, scale=recips[it][:, 0:1])
                    nc.sync.dma_start(
                        out=out_d[(sbi * NIT + it) * P:(sbi * NIT + it + 1) * P,
                                  ch * 512:(ch + 1) * 512],
                        in_=ob[:])

    nc.compile()
    return nc


_NC_CACHE = None


def kernel(x, wq, wk, wv, wo):
    global _NC_CACHE
    if _NC_CACHE is None:
        _NC_CACHE = _build()
    nc = _NC_CACHE
    core_ids = list(range(N_CORES))
    in_maps = []
    for b in range(N_CORES):
        in_maps.append({
            "x": np.ascontiguousarray(x[b], dtype=np.float32),
            "wq": np.ascontiguousarray(wq, dtype=np.float32),
            "wk": np.ascontiguousarray(wk, dtype=np.float32),
            "wv": np.ascontiguousarray(wv, dtype=np.float32),
            "wo": np.ascontiguousarray(wo, dtype=np.float32),
        })
    res = run_bass_kernel_spmd(nc, in_maps, core_ids)
    return np.stack([res.results[b]["out"] for b in range(N_CORES)], axis=0)


# revision 15
# speedup vs baseline: 1.0918x; 1.0918x over previous
"""Single-head encoder attention block on 8 Trainium2 NeuronCores.

Math (per batch element b):
    q = x @ wq.T ; k = x @ wk.T ; v = x @ wv.T
    scores = (q @ k.T) / sqrt(1024) ; attn = softmax(scores, -1)
    out = (attn @ v) @ wo.T

Sharding: data-parallel over batch — batch 8 maps 1:1 onto the 8 cores;
weights replicated. No collectives.

Per-core algorithm (storage fp32 bits; matmul operands typed MM_DT):
  Two weight-product folds remove almost all operand transposes:
      scores = x (wq.T wk) x.T / 32            M  := wq.T @ wk
      attn @ v @ wo.T = attn @ x @ (wo wv).T   via UT[d,do] = sum_vc wv[vc,d] woT[vc,do]
  Phase A (ordered so DMA/PE never idle at phase seams):
    A0: xT via identity-matmul transposes (resident)  [1024d, 2048s]
    A1: woT (identity-matmul transposes, do-halves), UT = wv-nat x woT,
        Z = xT-stationary x UT (resident)             [2048j, 1024do]
    A2: M = wq.T @ wk (wk resident, wq column-sliced) [1024d1, 1024d2]
    A3: F = (M @ xT)/32 spilled to DRAM               [1024d2, 2048i]
  Phase B (per i-superblock of SB=512):
    scoresT[j,i] = sum_d2 xT[d2,j]*F[d2,i]; expT = exp(scoresT)
    rowsum over j via ones-matmul; reciprocal; transposed to per-partition
    out[i,do] = (sum_j expT[j,i-tile] * Z[j,do]) * recip[i]   (expT stationary)
"""

import os
import sys

for _p in ("/opt/trn_rl_repo", "/root/.axon_site/_ro/trn_rl_repo"):
    if os.path.isdir(_p) and _p not in sys.path:
        sys.path.insert(0, _p)

import numpy as np
from contextlib import ExitStack

import concourse.bacc as bacc
import concourse.tile as tile
from concourse import mybir, masks
from concourse.bass_utils import run_bass_kernel_spmd

P = 128
S = 2048          # sequence length (per core)
D = 1024          # model dim = dk = dv
NS = S // P       # 16 seq tiles
ND = D // P       # 8 dim tiles
SB = 512          # i-superblock width (query columns per block)
NSB = S // SB     # 4 superblocks
NIT = SB // P     # 4 i-tiles per superblock
SCALE = 1.0 / 32.0  # 1/sqrt(1024)
N_CORES = 8

DT = mybir.dt.float32
MM_DT = mybir.dt.float32r if os.environ.get("ENC_MM_DT", "f32r") == "f32r" else mybir.dt.float32
F32 = mybir.dt.float32
EXP = mybir.ActivationFunctionType.Exp
COPY = mybir.ActivationFunctionType.Copy


def _build():
    nc = bacc.Bacc("TRN2", target_bir_lowering=False, debug=False, num_devices=N_CORES)

    x_in = nc.dram_tensor("x", [S, D], DT, kind="ExternalInput").ap()
    wq_in = nc.dram_tensor("wq", [D, D], DT, kind="ExternalInput").ap()
    wk_in = nc.dram_tensor("wk", [D, D], DT, kind="ExternalInput").ap()
    wv_in = nc.dram_tensor("wv", [D, D], DT, kind="ExternalInput").ap()
    wo_in = nc.dram_tensor("wo", [D, D], DT, kind="ExternalInput").ap()
    out_d = nc.dram_tensor("out", [S, D], DT, kind="ExternalOutput").ap()
    f_dram = nc.dram_tensor("f_scratch", [D, S], MM_DT).ap()

    mm = nc.tensor.matmul

    with tile.TileContext(nc) as tc, ExitStack() as top:
        cst = top.enter_context(tc.tile_pool(name="cst", bufs=1))
        ident_f32 = cst.tile([P, P], DT)
        masks.make_identity(nc, ident_f32[:])
        ident = cst.tile([P, P], MM_DT)
        nc.vector.tensor_copy(ident[:], ident_f32[:])
        ones_f32 = cst.tile([P, 1], DT)
        nc.gpsimd.memset(ones_f32[:], 1.0)
        ones = cst.tile([P, 1], MM_DT)
        nc.vector.tensor_copy(ones[:], ones_f32[:])

        def tr(out_ap, in_ap):
            """out_ap[PSUM 128x128] = in_ap.T via normal matmul against identity."""
            mm(out_ap, in_ap, ident[:], start=True, stop=True)

        res1 = top.enter_context(tc.tile_pool(name="res1", bufs=1))
        xt = res1.tile([P, ND * S], MM_DT)    # xT: tile d -> [:, d*S:(d+1)*S] = [d-part, s]
        res2 = top.enter_context(tc.tile_pool(name="res2", bufs=1))
        zres = res2.tile([P, NS * D], MM_DT)  # Z: tile j -> [:, j*D:(j+1)*D] = [j-part, do]

        # ---------------- Phase A0/A1: woT+UT first, xT, Z ----------------
        with ExitStack() as pw:
            tpps = pw.enter_context(tc.tile_pool(name="tpps", bufs=3, space="PSUM"))
            mmps = pw.enter_context(tc.tile_pool(name="mmps", bufs=5, space="PSUM"))
            ldp = pw.enter_context(tc.tile_pool(name="ldp", bufs=7))
            wvp = pw.enter_context(tc.tile_pool(name="wvp", bufs=1))
            hwork = pw.enter_context(tc.tile_pool(name="hwork", bufs=1))

            wvn = wvp.tile([P, ND * D], MM_DT)   # wv natural: vc-tile t -> [:, t*D:(t+1)*D]

            def load_wo_half(h):
                chunks = []
                for dot in range(4):
                    for hf in range(2):
                        wn = ldp.tile([P, 512], MM_DT, name=f"wo{h}{dot}{hf}", tag="ld")
                        do_row = h * 4 + dot
                        nc.sync.dma_start(
                            out=wn[:],
                            in_=wo_in[do_row * P:(do_row + 1) * P,
                                      hf * 512:(hf + 1) * 512].bitcast(MM_DT))
                        chunks.append(wn)
                return chunks

            def head_compute(h, wo_chunks):
                """woT half + UT half (no xT dependency — fills PE while x streams)."""
                wot_h = hwork.tile([P, ND * 512], MM_DT, name=f"woth{h}", tag="wot")
                for dot in range(4):
                    for vc in range(ND):
                        wn = wo_chunks[dot * 2 + vc // 4]
                        tp = tpps.tile([P, P], F32, tag="tp")
                        tr(tp[:], wn[:, (vc % 4) * P:(vc % 4 + 1) * P])
                        nc.vector.tensor_copy(
                            wot_h[:, vc * 512 + dot * P: vc * 512 + (dot + 1) * P], tp[:])
                ut_h = hwork.tile([P, ND * 512], MM_DT, name=f"uth{h}", tag="ut")
                for d in range(ND):
                    ps = mmps.tile([P, 512], F32, tag="mm")
                    for vc in range(ND):
                        mm(ps[:],
                           wvn[:, vc * D + d * P: vc * D + (d + 1) * P],
                           wot_h[:, vc * 512:(vc + 1) * 512],
                           start=(vc == 0), stop=(vc == ND - 1))
                    nc.scalar.copy(ut_h[:, d * 512:(d + 1) * 512], ps[:])
                return ut_h

            def do_half_z(h, ut_h):
                for j in range(NS):
                    ps = mmps.tile([P, 512], F32, tag="mm")
                    for d in range(ND):
                        mm(ps[:],
                           xt[:, d * S + j * P: d * S + (j + 1) * P],
                           ut_h[:, d * 512:(d + 1) * 512],
                           start=(d == 0), stop=(d == ND - 1))
                    nc.scalar.copy(zres[:, j * D + h * 512: j * D + (h + 1) * 512], ps[:])

            # emission order drives DMA priority: wo-h0 + wv on sync queues,
            # x chunks in parallel on the ACT hwdge queues
            wo0 = load_wo_half(0)
            for t in range(ND):
                nc.sync.dma_start(out=wvn[:, t * D:(t + 1) * D], in_=wv_in[t * P:(t + 1) * P, :].bitcast(MM_DT))
            x_chunks = []
            for s in range(NS):
                for hf in range(2):
                    xs = ldp.tile([P, 512], MM_DT, name=f"xs{s}{hf}", tag="ld")
                    nc.scalar.dma_start(
                        out=xs[:],
                        in_=x_in[s * P:(s + 1) * P, hf * 512:(hf + 1) * 512].bitcast(MM_DT))
                    x_chunks.append(xs)

            ut0 = head_compute(0, wo0)   # PE busy on wo transposes + UT while x streams

            # xT: dense transpose burst
            for s in range(NS):
                for d in range(ND):
                    xs = x_chunks[s * 2 + d // 4]
                    tp = tpps.tile([P, P], F32, tag="tp")
                    tr(tp[:], xs[:, (d % 4) * P:(d % 4 + 1) * P])
                    nc.vector.tensor_copy(xt[:, d * S + s * P: d * S + (s + 1) * P], tp[:])

            do_half_z(0, ut0)
            ut1 = head_compute(1, load_wo_half(1))
            do_half_z(1, ut1)

        # ---------------- Phase A2/A3: M then F ----------------
        # pool order: earliest-freed regions at the bottom so phase B's fbp
        # (opened first in pb) lands on space freed right after M
        with ExitStack() as pa:
            mmps2 = pa.enter_context(tc.tile_pool(name="mmps2", bufs=6, space="PSUM"))
            ldq = pa.enter_context(tc.tile_pool(name="ldq", bufs=4))
            wkp = pa.enter_context(tc.tile_pool(name="wkp", bufs=1))
            mwork = pa.enter_context(tc.tile_pool(name="mwork", bufs=1))
            evp = pa.enter_context(tc.tile_pool(name="evp", bufs=4))

            wkn = wkp.tile([P, ND * D], MM_DT)
            mres = mwork.tile([P, ND * D], MM_DT)  # M d1-tile -> [:, d1*D:(d1+1)*D] = [d1-part, d2]
            for t in range(ND):
                nc.sync.dma_start(out=wkn[:, t * D:(t + 1) * D], in_=wk_in[t * P:(t + 1) * P, :].bitcast(MM_DT))

            # A2: M = wq.T @ wk; wq streamed as [128, 256] column slices
            for q in range(4):           # d1-pairs
                pq = [mmps2.tile([P, 512], F32, name=f"mq{i}", tag="mm") for i in range(4)]
                for ct in range(ND):
                    wqs = ldq.tile([P, 256], MM_DT, tag="wq")
                    nc.sync.dma_start(
                        out=wqs[:],
                        in_=wq_in[ct * P:(ct + 1) * P, q * 256:(q + 1) * 256].bitcast(MM_DT))
                    for dl in range(2):
                        for ch in range(2):
                            mm(pq[dl * 2 + ch][:],
                               wqs[:, dl * P:(dl + 1) * P],
                               wkn[:, ct * D + ch * 512: ct * D + (ch + 1) * 512],
                               start=(ct == 0), stop=(ct == ND - 1))
                for dl in range(2):
                    for ch in range(2):
                        d1 = q * 2 + dl
                        nc.scalar.copy(mres[:, d1 * D + ch * 512: d1 * D + (ch + 1) * 512],
                                       pq[dl * 2 + ch][:])

            # A3: F[d2,i] = sum_d1 M[d1,d2] xT[d1,i], scaled by 1/32, spilled
            for d2 in range(ND):
                pss = [mmps2.tile([P, 512], F32, name=f"fps{ic}", tag="mm") for ic in range(4)]
                for d1 in range(ND):
                    for ic in range(4):
                        mm(pss[ic][:],
                           mres[:, d1 * D + d2 * P: d1 * D + (d2 + 1) * P],
                           xt[:, d1 * S + ic * 512: d1 * S + (ic + 1) * 512],
                           start=(d1 == 0), stop=(d1 == ND - 1))
                for ic in range(4):
                    ev = evp.tile([P, 512], MM_DT, tag="ev")
                    nc.scalar.mul(ev[:], pss[ic][:], SCALE)
                    nc.sync.dma_start(out=f_dram[d2 * P:(d2 + 1) * P, ic * 512:(ic + 1) * 512], in_=ev[:])

        # ---------------- Phase B ----------------
        with ExitStack() as pb:
            scps = pb.enter_context(tc.tile_pool(name="scps", bufs=3, space="PSUM"))
            outps = pb.enter_context(tc.tile_pool(name="outps", bufs=3, space="PSUM"))
            miscps = pb.enter_context(tc.tile_pool(name="miscps", bufs=2, space="PSUM"))
            fbp = pb.enter_context(tc.tile_pool(name="fbp", bufs=10))
            expp = pb.enter_context(tc.tile_pool(name="expp", bufs=16))
            outsb = pb.enter_context(tc.tile_pool(name="outsb", bufs=3))
            rsp = pb.enter_context(tc.tile_pool(name="rsp", bufs=2))
            rtp_pool = pb.enter_context(tc.tile_pool(name="rtp_pool", bufs=6))

            for sbi in range(NSB):
                fb = []
                for d2 in range(ND):
                    f = fbp.tile([P, SB], MM_DT, name=f"fb{d2}", tag="fb")
                    nc.sync.dma_start(out=f[:], in_=f_dram[d2 * P:(d2 + 1) * P, sbi * SB:(sbi + 1) * SB])
                    fb.append(f)

                # scoresT + exp per j-tile
                ets = []
                for j in range(NS):
                    sc = scps.tile([P, SB], F32, tag="sc")
                    for d2 in range(ND):
                        mm(sc[:],
                           xt[:, d2 * S + j * P: d2 * S + (j + 1) * P],
                           fb[d2][:],
                           start=(d2 == 0), stop=(d2 == ND - 1))
                    et = expp.tile([P, SB], MM_DT, name=f"et{j}", tag="et")
                    nc.scalar.activation(et[:], sc[:], EXP)
                    ets.append(et)

                # rowsums over j (partition dim) via ones-matmul
                rs = miscps.tile([1, SB], F32, tag="m")
                for j in range(NS):
                    mm(rs[:], ones[:, 0:1], ets[j][:], start=(j == 0), stop=(j == NS - 1))

                # reciprocal chain (DVE) — emitted early so it overlaps out-MMs
                rs_sb = rsp.tile([1, SB], DT, tag="rs")
                nc.vector.tensor_copy(rs_sb[:], rs[:])
                rc_sb = rsp.tile([1, SB], DT, tag="rc")
                nc.vector.reciprocal(rc_sb[:], rs_sb[:])

                # out[i,do] = sum_j expT[j, i-tile].T @ Z[j, do-chunk]; evict fused
                recips = [None] * NIT
                for gi in range(NIT * 2):
                    it, ch = gi // 2, gi % 2
                    op = outps.tile([P, 512], F32, name=f"op{ch}", tag="op")
                    for j in range(NS):
                        mm(op[:],
                           ets[j][:, it * P:(it + 1) * P],
                           zres[:, j * D + ch * 512: j * D + (ch + 1) * 512],
                           start=(j == 0), stop=(j == NS - 1))
                    if gi == 0:
                        # per-partition recip tiles via tiny PE transposes; PE
                        # reaches these after group 0 while DVE chain is done
                        for it2 in range(NIT):
                            tp = miscps.tile([P, 1], F32, name=f"rtp{it2}", tag="m")
                            nc.tensor.transpose(tp[:], rc_sb[:1, it2 * P:(it2 + 1) * P], ident_f32[:1, :1])
                            rt = rtp_pool.tile([P, 1], DT, name=f"rt{it2}", tag="rt")
                            nc.vector.tensor_copy(rt[:], tp[:])
                            recips[it2] = rt
                    ob = outsb.tile([P, 512], DT, tag="ob")
                    nc.scalar.activation(ob[:], op[:], COPY, scale=recips[it][:, 0:1])
                    nc.sync.dma_start(
                        out=out_d[(sbi * NIT + it) * P:(sbi * NIT + it + 1) * P,
                                  ch * 512:(ch + 1) * 512],
                        in_=ob[:])

    nc.compile()
    return nc


_NC_CACHE = None


def kernel(x, wq, wk, wv, wo):
    global _NC_CACHE
    if _NC_CACHE is None:
        _NC_CACHE = _build()
    nc = _NC_CACHE
    core_ids = list(range(N_CORES))
    in_maps = []
    for b in range(N_CORES):
        in_maps.append({
            "x": np.ascontiguousarray(x[b], dtype=np.float32),
            "wq": np.ascontiguousarray(wq, dtype=np.float32),
            "wk": np.ascontiguousarray(wk, dtype=np.float32),
            "wv": np.ascontiguousarray(wv, dtype=np.float32),
            "wo": np.ascontiguousarray(wo, dtype=np.float32),
        })
    res = run_bass_kernel_spmd(nc, in_maps, core_ids)
    return np.stack([res.results[b]["out"] for b in range(N_CORES)], axis=0)
